# revision 51
# baseline (speedup 1.0000x reference)
"""Trainium2 Bass kernel for causal multi-head attention (dense transformer block).

Reference computation (per problem statement):
    qkv = x @ Wqkv.T ; split q,k,v ; RoPE(q), RoPE(k)
    scores = q @ k.T / sqrt(dh), causal mask, softmax
    o = probs @ v ; out = o @ Wo.T

Sharding: batch*heads across 8 cores (core c: batch c//4, heads 4*(c%4)..+4).
Each core computes its partial contribution to out (its heads through the
output projection); the host sums the 4 partials per batch at unshard time.
No device collectives (they are latency-bound disasters at these sizes).

Device-side layout strategy (per core):
  - host passes x.T, Wq/Wk rows permuted (even dims then odd dims per head,
    q pre-scaled by 1/sqrt(dh)), all bf16
  - qT/kT computed in [dims, seq] layout -> RoPE on DVE with host cos/sin
    tables -> scores^T tiles [j,i] via row-packed K=32 matmuls (4 heads
    concurrently in the PE array)
  - exp on ScalarE (no max subtraction: scores are O(1) by construction),
    softmax denominators via col-packed M=1 ones-matmuls, probs kept bf16
  - PV via col-packed matmuls -> o^T ; normalization via selector-matmul
    partition-broadcast of the sums + reciprocal ; output projection ->
    partial out

Scheduling (the part that matters for speed):
  - PE warmup matmuls during the input-DMA wait (HAM clock gate)
  - batched big-tile input DMAs ordered by first use
  - prefix projects all q + k/v for block 0; k/v for blocks 1-3 are
    pipelined through the score PSUM ring as quarter-units during the
    preceding block, with ScalarE PSUM->SBUF staging casts and the rope
    spread over later iterations
  - in the attention loop, pv/row-sum matmuls lag scores/exp by 2
    iterations and output-projection units pop at jt>=3 so the PE FIFO
    never head-of-line blocks on ScalarE/DVE results; each block's norm
    chain is emitted after the next block's first scores
"""

import os
import numpy as np
import ml_dtypes

B, S, D = 2, 2048, 1024
H, DH = 16, 64
HALF = DH // 2            # 32
NCORES = 8
GPB = 4                   # cores (head-groups) per batch
HPC = H // GPB            # 4 heads per core
THETA = 10000.0
IB = 512                  # query block (free dim of scores^T)
NIB = S // IB             # 4
JT = 128                  # key tile (partition dim of scores^T)
NJT = S // JT             # 16
KT = 128                  # contraction tile
NKT = D // KT             # 8

BF16 = ml_dtypes.bfloat16

_NC_CACHE = None


def _build_nc():
    """Build + compile the (SPMD-identical) single-core Bass graph once."""
    global _NC_CACHE
    if _NC_CACHE is not None:
        return _NC_CACHE

    import concourse.bass as bass
    import concourse.mybir as mybir
    import concourse.tile as tile
    from concourse import bacc

    dt = mybir.dt
    f32 = dt.float32
    bf = dt.bfloat16
    EXP = mybir.ActivationFunctionType.Exp

    nc = bacc.Bacc("TRN2", target_bir_lowering=False, debug=False,
                   enable_asserts=False)

    xT = nc.dram_tensor("xT", [D, S], bf, kind="ExternalInput").ap()
    wqk = nc.dram_tensor("wqk", [D, 4 * HPC * HALF], bf, kind="ExternalInput").ap()
    wv = nc.dram_tensor("wv", [D, HPC * DH], bf, kind="ExternalInput").ap()
    wo = nc.dram_tensor("wo", [HPC * DH, D], bf, kind="ExternalInput").ap()
    cosT = nc.dram_tensor("cosT", [HPC * HALF, S], bf, kind="ExternalInput").ap()
    sinT = nc.dram_tensor("sinT", [HPC * HALF, S], bf, kind="ExternalInput").ap()
    maskm = nc.dram_tensor("maskm", [JT, 2 * JT], bf, kind="ExternalInput").ap()
    selm = nc.dram_tensor("selm", [128, 256], bf, kind="ExternalInput").ap()
    out = nc.dram_tensor("out", [S, D], bf, kind="ExternalOutput").ap()

    with tile.TileContext(nc) as tc:
        _body(nc, tc, mybir, bass,
              xT, wqk, wv, wo, cosT, sinT, maskm, selm, out)

    nc.compile()
    _NC_CACHE = nc
    return nc


def _body(nc, tc, mybir, bass, xT, wqk, wv, wo, cosT, sinT, maskm, selm, out):
    dt = mybir.dt
    f32 = dt.float32
    bf = dt.bfloat16
    EXP = mybir.ActivationFunctionType.Exp

    from contextlib import ExitStack
    ctx = ExitStack()
    with ctx:
        consts = ctx.enter_context(tc.tile_pool(name="consts", bufs=1))
        persist = ctx.enter_context(tc.tile_pool(name="persist", bufs=1))
        ropet = ctx.enter_context(tc.tile_pool(name="ropet", bufs=6))
        prpool = ctx.enter_context(tc.tile_pool(name="prpool", bufs=8))
        recpool = ctx.enter_context(tc.tile_pool(name="recpool", bufs=2))
        stpool = ctx.enter_context(tc.tile_pool(name="stpool", bufs=4))
        # PSUM budget: big(2 slots x 2 banks) + acc(2 x 1) + rs(1) + op(1) = 8
        psbig = ctx.enter_context(tc.tile_pool(name="psbig", bufs=2, space="PSUM"))
        psacc = ctx.enter_context(tc.tile_pool(name="psacc", bufs=2, space="PSUM"))
        psrs = ctx.enter_context(tc.tile_pool(name="psrs", bufs=1, space="PSUM"))
        psop = ctx.enter_context(tc.tile_pool(name="psop", bufs=1, space="PSUM"))

        # ---- PE warmup: run throwaway matmuls during the input-DMA wait so
        # the HAM clock gate is at 8/8 (2.4 GHz) by the time real work lands.
        warm = consts.tile([128, 512], bf, tag="warm")
        nc.vector.memset(warm, 0.0)
        wps = psop.tile([128, 512], f32, tag="op", name="warmps")
        for _ in range(14):
            nc.tensor.matmul(wps, lhsT=warm[:, 0:128], rhs=warm,
                             start=True, stop=True)

        # ---- constant/persistent SBUF loads -------------------------------
        # Batched big-tile DMAs (the per-DMA issue on the sync queue costs
        # ~0.6us, so few big transfers beat many small ones), ordered so the
        # first projection chunk can start as early as possible.
        xr = xT.rearrange("(k p) s -> p k s", k=NKT)
        xt_all = consts.tile([128, NKT, S], bf, tag="xt")
        wqk_all = consts.tile([128, NKT, 4 * HPC * HALF], bf, tag="wqk")
        wv_all = consts.tile([128, NKT, HPC * DH], bf, tag="wv")
        wo_all = consts.tile([128, 2, D], bf, tag="wo")
        cos = consts.tile([128, S], bf, tag="cos")
        sin = consts.tile([128, S], bf, tag="sin")
        maskt = consts.tile([JT, 2, JT], bf, tag="maskt")
        selt = consts.tile([128, 256], bf, tag="selt")
        # transfer order follows first use: the k-halves of x chunk 2 and
        # wqk first (the prefix projects q2 first; see below), cos/sin next
        # (rope chain), then the remaining x chunks in projection order
        wqr = wqk.rearrange("(k p) f -> p k f", k=NKT)
        h = NKT // 2
        c2 = slice(2 * IB, 3 * IB)
        nc.sync.dma_start(out=xt_all[:, 0:h, c2], in_=xr[:, 0:h, c2])
        nc.sync.dma_start(out=wqk_all[:, 0:h, :], in_=wqr[:, 0:h, :])
        nc.sync.dma_start(out=xt_all[:, h:NKT, c2], in_=xr[:, h:NKT, c2])
        nc.sync.dma_start(out=wqk_all[:, h:NKT, :], in_=wqr[:, h:NKT, :])
        nc.sync.dma_start(out=cos, in_=cosT)
        nc.sync.dma_start(out=sin, in_=sinT)
        for c in (3, 1, 0):
            csl = slice(c * IB, (c + 1) * IB)
            nc.sync.dma_start(out=xt_all[:, :, csl], in_=xr[:, :, csl])
        nc.sync.dma_start(out=wv_all,
                            in_=wv.rearrange("(k p) f -> p k f", k=NKT))
        nc.sync.dma_start(out=wo_all,
                            in_=wo.rearrange("(k p) f -> p k f", k=2))
        nc.sync.dma_start(out=maskt,
                            in_=maskm.rearrange("p (h f) -> p h f", h=2))
        nc.sync.dma_start(out=selt, in_=selm)
        ones = consts.tile([128, 1], bf, tag="ones")
        nc.vector.memset(ones, 1.0)
        xt = [xt_all[:, k, :] for k in range(NKT)]
        wqkt = [wqk_all[:, k, :] for k in range(NKT)]
        wvt = [wv_all[:, k, :] for k in range(NKT)]
        wot = [wo_all[:, k, :] for k in range(2)]

        # persistent activations
        qR = [persist.tile([128, S], bf, tag=f"qR{i}", name=f"qR{i}") for i in range(2)]
        kR = [persist.tile([128, S], bf, tag=f"kR{i}", name=f"kR{i}") for i in range(2)]
        vbuf = persist.tile([128, NJT, HPC * DH], bf, tag="vbuf")
        oT = [persist.tile([128, S], bf, tag=f"oT{i}", name=f"oT{i}") for i in range(2)]

        # staging buffers for pipelined q/k projection (cast out of PSUM
        # fast, rope later from SBUF so the score ring slot frees quickly)
        kst = persist.tile([128, 4, 256], bf, tag="kst")
        kstf = kst.rearrange("p g f -> p (g f)")
        qst = persist.tile([128, 4, 256], bf, tag="qst")
        qstf = qst.rearrange("p g f -> p (g f)")

        # ---- projection emitters ------------------------------------------
        # wqk cols: [0:128]=q even dims (x1), [128:256]=q odd (x2),
        #           [256:384]=k even, [384:512]=k odd  (4 heads x 32 each)
        def emit_proj_full(ic, r1, r2, fbase):
            """Prefix-time q/k projection + rope straight from PSUM."""
            isl = slice(ic * IB, (ic + 1) * IB)
            pq = psbig.tile([128, 2 * IB], f32, tag="big", name="pq")
            for half in range(2):
                fo = fbase + half * 128
                for k in range(NKT):
                    nc.tensor.matmul(
                        pq[:, half * IB:(half + 1) * IB],
                        lhsT=wqkt[k][:, fo:fo + 128],
                        rhs=xt[k][:, isl],
                        start=(k == 0), stop=(k == NKT - 1))
            # rope: r1 = x1*cos - x2*sin ; r2 = x1*sin + x2*cos
            t1 = ropet.tile([128, IB], bf, tag="rt")
            t2 = ropet.tile([128, IB], bf, tag="rt")
            t3 = ropet.tile([128, IB], bf, tag="rt")
            t4 = ropet.tile([128, IB], bf, tag="rt")
            nc.vector.tensor_mul(t1, pq[:, 0:IB], cos[:, isl])
            nc.vector.tensor_mul(t2, pq[:, IB:2 * IB], sin[:, isl])
            nc.vector.tensor_mul(t3, pq[:, 0:IB], sin[:, isl])
            nc.vector.tensor_mul(t4, pq[:, IB:2 * IB], cos[:, isl])
            nc.vector.tensor_sub(r1[:, isl], t1, t2)
            nc.vector.tensor_add(r2[:, isl], t3, t4)

        def emit_proj_v(jt, pool):
            """One v j-tile ([seq, dim] layout) through the given PSUM pool.
            The PSUM->SBUF copy rides the ScalarE queue (idle slack there;
            the DVE queue would delay the ring-slot release)."""
            vp = pool.tile([128, 2 * IB] if pool is psbig else [128, IB],
                           f32, tag="big" if pool is psbig else "acc",
                           name="vp")
            for k in range(NKT):
                nc.tensor.matmul(
                    vp[:, 0:HPC * DH],
                    lhsT=xt[k][:, jt * 128:(jt + 1) * 128],
                    rhs=wvt[k],
                    start=(k == 0), stop=(k == NKT - 1))
            nc.scalar.copy(out=vbuf[:, jt, :], in_=vp[:, 0:HPC * DH])

        def emit_proj_quarter(c, g, which):
            """In-loop q/k-projection quarter: 8 matmuls into a score-ring
            slot plus a fast ScalarE cast to the SBUF staging tile."""
            stage, fbase = (qst, 0) if which == "q" else (kst, 256)
            half, part = g // 2, g % 2
            fo = fbase + half * 128
            q_lo = c * IB + part * 256
            pq = psbig.tile([128, 2 * IB], f32, tag="big", name="pqk")
            for k in range(NKT):
                nc.tensor.matmul(
                    pq[:, 0:256],
                    lhsT=wqkt[k][:, fo:fo + 128],
                    rhs=xt[k][:, q_lo:q_lo + 256],
                    start=(k == 0), stop=(k == NKT - 1))
            nc.scalar.copy(out=stage[:, g, :], in_=pq[:, 0:256])

        def emit_rope_pipe(c, step, which):
            """One third of the rope for a pipelined q/k chunk (spread
            across iterations to avoid DVE-queue bursts)."""
            stf = qstf if which == "q" else kstf
            r1, r2 = (qR[0], qR[1]) if which == "q" else (kR[0], kR[1])
            hold = rope_hold[which]
            isl = slice(c * IB, (c + 1) * IB)
            x1 = stf[:, 0:IB]
            x2 = stf[:, IB:2 * IB]
            if step == 0:
                t1 = ropet.tile([128, IB], bf, tag="rt")
                t2 = ropet.tile([128, IB], bf, tag="rt")
                nc.vector.tensor_mul(t1, x1, cos[:, isl])
                nc.vector.tensor_mul(t2, x2, sin[:, isl])
                hold[0:2] = [t1, t2]
            elif step == 1:
                t3 = ropet.tile([128, IB], bf, tag="rt")
                t4 = ropet.tile([128, IB], bf, tag="rt")
                nc.vector.tensor_mul(t3, x1, sin[:, isl])
                nc.vector.tensor_mul(t4, x2, cos[:, isl])
                hold[2:4] = [t3, t4]
            else:
                nc.vector.tensor_sub(r1[:, isl], hold[0], hold[1])
                nc.vector.tensor_add(r2[:, isl], hold[2], hold[3])
        rope_hold = {"q": [None] * 4, "k": [None] * 4}

        # ---- prefix: all q chunks + k/v for block 0 -----------------------
        # chunk order q2,q3,q1,q0,k0: the score ring's first two slots then
        # WAR on the ropes of q0/k0 (needed for block 0's data anyway)
        # instead of on unrelated late-chunk ropes
        for ic in (2, 3, 1, 0):
            emit_proj_full(ic, qR[0], qR[1], 0)
        emit_proj_full(0, kR[0], kR[1], 256)
        for jt in range(4):
            emit_proj_v(jt, psacc)

        # ---- phase 2: attention per query block ---------------------------
        # output-projection units are spread one-per-j-iteration into the
        # NEXT block's attention loop so they fill PE gaps instead of
        # forming a serial chain at block boundaries
        pending_op = []

        def emit_op(ic, mc, pool=None, tag="op"):
            icsl = slice(ic * 128, (ic + 1) * 128)
            msl = slice(mc * 512, (mc + 1) * 512)
            op = (pool or psop).tile([128, 512], f32, tag=tag, name="op")
            nc.tensor.matmul(op, lhsT=oT[0][:, icsl], rhs=wot[0][:, msl],
                             start=True, stop=False)
            nc.tensor.matmul(op, lhsT=oT[1][:, icsl], rhs=wot[1][:, msl],
                             start=False, stop=True)
            st = stpool.tile([128, 512], bf, tag="st", name="st")
            nc.vector.tensor_copy(st, op)
            nc.sync.dma_start(out=out[icsl, msl], in_=st)

        def emit_scores(b, jt):
            """Scores matmuls + exp for one (query-block, key-tile) pair."""
            isl = slice(b * IB, (b + 1) * IB)
            jsl = slice(jt * JT, (jt + 1) * JT)
            sc1 = psbig.tile([128, 2 * IB], f32, tag="big", name="sc1")
            sc2 = psbig.tile([128, 2 * IB], f32, tag="big", name="sc2")
            for ph in range(2):
                for h in range(4):
                    dst = sc1 if h < 2 else sc2
                    col = (h % 2) * IB
                    nc.tensor.matmul(
                        dst[:, col:col + IB],
                        lhsT=kR[ph][32 * h:32 * h + 32, jsl],
                        rhs=qR[ph][32 * h:32 * h + 32, isl],
                        start=(ph == 0), stop=(ph == 1),
                        tile_position=(32 * h, 0))
            pr1 = prpool.tile([128, 2, IB], bf, tag="pr", name="pr1")
            pr2 = prpool.tile([128, 2, IB], bf, tag="pr", name="pr2")
            delta = jt * JT - b * IB
            for (sc, pr) in ((sc1, pr1), (sc2, pr2)):
                scv = sc.rearrange("p (h f) -> p h f", h=2)
                if delta < 0:
                    nc.scalar.activation(pr, sc, EXP)
                else:
                    # exp only the possibly-valid region (pv/rs matmuls
                    # read the same subrange, so the rest of pr can stay
                    # garbage), mask the 128-wide diagonal block
                    nc.scalar.activation(pr[:, :, delta:IB],
                                         scv[:, :, delta:IB], EXP)
                    nc.vector.tensor_mul(pr[:, :, delta:delta + JT],
                                         pr[:, :, delta:delta + JT],
                                         maskt)
            return pr1, pr2

        def emit_pv(b, jt, pr1, pr2, pv1, pv2, rs, last):
            """PV + row-sum matmuls for one (b, jt); lags scores by 1 iter."""
            vj = vbuf[:, jt, :]
            st0 = (jt == 0)
            d0 = max(jt * JT - b * IB, 0)
            nc.tensor.matmul(pv1[0:64, d0:IB], lhsT=vj[:, 0:64],
                             rhs=pr1[:, 0, d0:IB], start=st0, stop=last,
                             skip_group_check=True, tile_position=(0, 0))
            nc.tensor.matmul(pv1[64:128, d0:IB], lhsT=vj[:, 64:128],
                             rhs=pr1[:, 1, d0:IB], start=st0, stop=last,
                             skip_group_check=True, tile_position=(0, 64))
            nc.tensor.matmul(pv2[0:64, d0:IB], lhsT=vj[:, 128:192],
                             rhs=pr2[:, 0, d0:IB], start=st0, stop=last,
                             skip_group_check=True, tile_position=(0, 0))
            nc.tensor.matmul(pv2[64:128, d0:IB], lhsT=vj[:, 192:256],
                             rhs=pr2[:, 1, d0:IB], start=st0, stop=last,
                             skip_group_check=True, tile_position=(0, 64))
            nc.tensor.matmul(rs[0:1, d0:IB], lhsT=ones,
                             rhs=pr1[:, 0, d0:IB], start=st0, stop=last,
                             skip_group_check=True, tile_position=(0, 0))
            nc.tensor.matmul(rs[32:33, d0:IB], lhsT=ones,
                             rhs=pr1[:, 1, d0:IB], start=st0, stop=last,
                             skip_group_check=True, tile_position=(0, 32))
            nc.tensor.matmul(rs[64:65, d0:IB], lhsT=ones,
                             rhs=pr2[:, 0, d0:IB], start=st0, stop=last,
                             skip_group_check=True, tile_position=(0, 64))
            nc.tensor.matmul(rs[96:97, d0:IB], lhsT=ones,
                             rhs=pr2[:, 1, d0:IB], start=st0, stop=last,
                             skip_group_check=True, tile_position=(0, 96))

        def emit_norm(b, pv1, pv2, rs):
            """Normalization chain: oT = pv * broadcast(1/rs).

            Cast the sum rows to bf16 SBUF (rows 0/32/64/96 hold sums, the
            rest the memset 1.0s), selector-matmul broadcasts r across
            partitions into PSUM (bc1 rows 0-63 <- rs row 0, rows 64-127 <-
            rs row 32, etc.), then reciprocal PSUM->SBUF so the final DVE
            multiply reads only one PSUM operand (pv)."""
            isl = slice(b * IB, (b + 1) * IB)
            rsbf = recpool.tile([128, IB], bf, tag="rsbf")
            nc.vector.tensor_copy(rsbf, rs)
            bc1 = psop.tile([128, IB], f32, tag="op", name="bc1")
            nc.tensor.matmul(bc1, lhsT=selt[:, 0:128], rhs=rsbf,
                             start=True, stop=True)
            bc2 = psrs.tile([128, IB], f32, tag="rs", name="bc2")
            nc.tensor.matmul(bc2, lhsT=selt[:, 128:256], rhs=rsbf,
                             start=True, stop=True)
            rec1 = recpool.tile([128, IB], f32, tag="rec")
            rec2 = recpool.tile([128, IB], f32, tag="rec")
            nc.vector.reciprocal_approx_fast(out=rec1, in_=bc1)
            nc.vector.reciprocal_approx_fast(out=rec2, in_=bc2)
            nc.vector.tensor_mul(oT[0][:, isl], pv1, rec1)
            nc.vector.tensor_mul(oT[1][:, isl], pv2, rec2)
            pending_op.extend((ic, mc) for ic in range(4 * b, 4 * b + 4)
                              for mc in range(2))

        # The jt loop lags pv/rs one iteration behind scores/exp so the PE
        # FIFO always holds runnable score matmuls while the ScalarE works:
        # PE order per iter is [scores(jt), pv(jt-1), op-unit, proj-inserts],
        # and each block's norm chain + accumulator allocation happen after
        # the next block's first scores are already in flight.  k/v
        # projection for block b+1 is pipelined through the score ring as
        # small quarter-units during block b (their column ranges of kR/vbuf
        # are disjoint from what block b reads).
        pend_pvs = []
        norm_args = None
        pv1 = pv2 = rs = None
        ins_backlog = []
        for b in range(NIB):
            njt = (b + 1) * (IB // JT)
            if b + 1 < NIB:
                ins_backlog += ([("pq", b + 1, g, "k") for g in range(4)]
                                + [("pr", b + 1, s, "k") for s in range(3)]
                                + [("v", 4 * (b + 1) + g) for g in range(4)])
            for jt in range(njt):
                pr1, pr2 = emit_scores(b, jt)
                if jt == 0:
                    # block boundary: flush all lagged pv groups of the
                    # previous block, then its norm chain
                    for p in pend_pvs:
                        emit_pv(**p)
                    pend_pvs.clear()
                    if norm_args is not None:
                        emit_norm(*norm_args)
                        norm_args = None
                elif len(pend_pvs) >= 2:
                    emit_pv(**pend_pvs.pop(0))
                if jt == 0:
                    # allocate this block's accumulators only after the
                    # previous block's norm (ring-slot order: rs -> bc2 ->
                    # next rs)
                    pv1 = psacc.tile([128, IB], f32, tag="acc", name="pv1")
                    pv2 = psacc.tile([128, IB], f32, tag="acc", name="pv2")
                    rs = psrs.tile([128, IB], f32, tag="rs", name="rs")
                    nc.vector.memset(rs, 1.0)
                # output-projection pops start at jt>=3 so they never
                # FIFO-block on the previous block's norm chain; block 3
                # keeps 4 units back so the drain has runnable PE work
                # during the final norm chain
                min_keep = 4 if b == NIB - 1 else 0
                if len(pending_op) > min_keep and jt >= 3:
                    emit_op(*pending_op.pop(0))
                    if (len(pending_op) > (njt - 1 - jt)
                            and len(pending_op) > min_keep):
                        emit_op(*pending_op.pop(0))
                # projection inserts, spread over the block's iterations
                if jt > 0 and ins_backlog:
                    iters_left = njt - jt
                    n_emit = -(-len(ins_backlog) // iters_left)  # ceil
                    for _ in range(n_emit):
                        ins = ins_backlog.pop(0)
                        if ins[0] == "pq":
                            emit_proj_quarter(ins[1], ins[2], ins[3])
                        elif ins[0] == "pr":
                            emit_rope_pipe(ins[1], ins[2], ins[3])
                        else:
                            emit_proj_v(ins[1], psbig)
                pend_pvs.append(dict(b=b, jt=jt, pr1=pr1, pr2=pr2,
                                     pv1=pv1, pv2=pv2, rs=rs,
                                     last=(jt == njt - 1)))
            norm_args = (b, pv1, pv2, rs)
        for p in pend_pvs:
            emit_pv(**p)
        pend_pvs.clear()
        # the 4 held-back block-2 units are runnable immediately: they keep
        # the PE streaming while the final norm chain runs on the DVE
        for _ in range(min(4, len(pending_op))):
            emit_op(*pending_op.pop(0))
        emit_norm(*norm_args)

        # drain the last block's output projection, alternating PSUM slots
        # (the rs bank is free by now) so the tail pipelines
        di = 0
        while pending_op:
            ic, mc = pending_op.pop(0)
            if di % 2 == 0:
                emit_op(ic, mc)
            else:
                emit_op(ic, mc, pool=psrs, tag="rs")
            di += 1


# ---------------------------------------------------------------------------
# Host-side sharding / unsharding
# ---------------------------------------------------------------------------

def _core_inputs(x, Wqkv, Wo, core):
    """Build the bf16 input map for one core (numpy, cheap)."""
    b = core // GPB
    heads = [HPC * (core % GPB) + j for j in range(HPC)]

    Wq = Wqkv[0 * D:1 * D]
    Wk = Wqkv[1 * D:2 * D]
    Wv = Wqkv[2 * D:3 * D]

    rows_x1 = [h * DH + 2 * t for h in heads for t in range(HALF)]
    rows_x2 = [h * DH + 2 * t + 1 for h in heads for t in range(HALF)]
    rows_v = [h * DH + d for h in heads for d in range(DH)]

    scale = 1.0 / np.sqrt(DH)
    wqk_host = np.concatenate([
        Wq[rows_x1] * scale, Wq[rows_x2] * scale,
        Wk[rows_x1], Wk[rows_x2],
    ], axis=0)                                   # [512, 1024]

    inv = THETA ** (-np.arange(HALF, dtype=np.float64) / HALF)
    ang = np.arange(S, dtype=np.float64)[None, :] * inv[:, None]   # [32, S]
    cos = np.tile(np.cos(ang), (HPC, 1))
    sin = np.tile(np.sin(ang), (HPC, 1))

    tri = (np.arange(JT)[None, :] >= np.arange(JT)[:, None]).astype(np.float32)
    maskm = np.tile(tri, (1, 2))                                   # [128, 256]

    # selector matrices for the partition-broadcast of the softmax
    # denominators: bc[p] = rec[sel_row(p)]
    selm = np.zeros((128, 256), dtype=np.float32)
    selm[0, 0:64] = 1.0       # bc1 rows 0-63   <- rec row 0  (head 0)
    selm[32, 64:128] = 1.0    # bc1 rows 64-127 <- rec row 32 (head 1)
    selm[64, 128:192] = 1.0   # bc2 rows 0-63   <- rec row 64 (head 2)
    selm[96, 192:256] = 1.0   # bc2 rows 64-127 <- rec row 96 (head 3)

    c = lambda a: np.ascontiguousarray(a).astype(BF16)
    return {
        "xT": c(x[b].T),
        "wqk": c(wqk_host.T),
        "wv": c(Wv[rows_v].T),
        "wo": c(Wo[:, rows_v].T),
        "cosT": c(cos),
        "sinT": c(sin),
        "maskm": c(maskm),
        "selm": c(selm),
    }


def _run(x, Wqkv, Wo, trace=False):
    nc = _build_nc()
    from concourse.bass_utils import run_bass_kernel_spmd
    in_maps = [_core_inputs(x, Wqkv, Wo, c) for c in range(NCORES)]
    res = run_bass_kernel_spmd(nc, in_maps, core_ids=list(range(NCORES)),
                               trace=trace)
    parts = [res.results[i]["out"].astype(np.float32) for i in range(NCORES)]
    full = np.stack([sum(parts[0:GPB]), sum(parts[GPB:2 * GPB])], axis=0)
    return full, res


def kernel(x, Wqkv, Wo):
    x = np.asarray(x, dtype=np.float32)
    Wqkv = np.asarray(Wqkv, dtype=np.float32)
    Wo = np.asarray(Wo, dtype=np.float32)
    full, _ = _run(x, Wqkv, Wo, trace=False)
    return full



# revision 53
# speedup vs baseline: 1.0334x; 1.0334x over previous
"""Trainium2 Bass kernel for causal multi-head attention (dense transformer block).

Reference computation (per problem statement):
    qkv = x @ Wqkv.T ; split q,k,v ; RoPE(q), RoPE(k)
    scores = q @ k.T / sqrt(dh), causal mask, softmax
    o = probs @ v ; out = o @ Wo.T

Sharding: batch*heads across 8 cores (core c: batch c//4, heads 4*(c%4)..+4).
Each core computes its partial contribution to out (its heads through the
output projection); the host sums the 4 partials per batch at unshard time.
No device collectives (they are latency-bound disasters at these sizes).

Device-side layout strategy (per core):
  - host passes x.T, Wq/Wk rows permuted (even dims then odd dims per head,
    q pre-scaled by 1/sqrt(dh)), all bf16
  - qT/kT computed in [dims, seq] layout -> RoPE on DVE with host cos/sin
    tables -> scores^T tiles [j,i] via row-packed K=32 matmuls (4 heads
    concurrently in the PE array)
  - exp on ScalarE (no max subtraction: scores are O(1) by construction),
    softmax denominators via col-packed M=1 ones-matmuls, probs kept bf16
  - PV via col-packed matmuls -> o^T ; normalization via selector-matmul
    partition-broadcast of the sums + reciprocal ; output projection ->
    partial out

Scheduling (the part that matters for speed):
  - PE warmup matmuls during the input-DMA wait (HAM clock gate)
  - batched big-tile input DMAs ordered by first use
  - prefix projects all q + k/v for block 0; k/v for blocks 1-3 are
    pipelined through the score PSUM ring as quarter-units during the
    preceding block, with ScalarE PSUM->SBUF staging casts and the rope
    spread over later iterations
  - in the attention loop, pv/row-sum matmuls lag scores/exp by 2
    iterations and output-projection units pop at jt>=3 so the PE FIFO
    never head-of-line blocks on ScalarE/DVE results; each block's norm
    chain is emitted after the next block's first scores
"""

import os
import numpy as np
import ml_dtypes

B, S, D = 2, 2048, 1024
H, DH = 16, 64
HALF = DH // 2            # 32
NCORES = 8
GPB = 4                   # cores (head-groups) per batch
HPC = H // GPB            # 4 heads per core
THETA = 10000.0
IB = 512                  # query block (free dim of scores^T)
NIB = S // IB             # 4
JT = 128                  # key tile (partition dim of scores^T)
NJT = S // JT             # 16
KT = 128                  # contraction tile
NKT = D // KT             # 8

BF16 = ml_dtypes.bfloat16

_NC_CACHE = None


def _build_nc():
    """Build + compile the (SPMD-identical) single-core Bass graph once."""
    global _NC_CACHE
    if _NC_CACHE is not None:
        return _NC_CACHE

    import concourse.bass as bass
    import concourse.mybir as mybir
    import concourse.tile as tile
    from concourse import bacc

    dt = mybir.dt
    f32 = dt.float32
    bf = dt.bfloat16
    EXP = mybir.ActivationFunctionType.Exp

    nc = bacc.Bacc("TRN2", target_bir_lowering=False, debug=False,
                   enable_asserts=False)

    xT = nc.dram_tensor("xT", [D, S], bf, kind="ExternalInput").ap()
    wqk = nc.dram_tensor("wqk", [D, 4 * HPC * HALF], bf, kind="ExternalInput").ap()
    wv = nc.dram_tensor("wv", [D, HPC * DH], bf, kind="ExternalInput").ap()
    wo = nc.dram_tensor("wo", [HPC * DH, D], bf, kind="ExternalInput").ap()
    cosT = nc.dram_tensor("cosT", [HPC * HALF, S], bf, kind="ExternalInput").ap()
    sinT = nc.dram_tensor("sinT", [HPC * HALF, S], bf, kind="ExternalInput").ap()
    maskm = nc.dram_tensor("maskm", [JT, 2 * JT], bf, kind="ExternalInput").ap()
    selm = nc.dram_tensor("selm", [128, 256], bf, kind="ExternalInput").ap()
    out = nc.dram_tensor("out", [S, D], bf, kind="ExternalOutput").ap()

    with tile.TileContext(nc) as tc:
        _body(nc, tc, mybir, bass,
              xT, wqk, wv, wo, cosT, sinT, maskm, selm, out)

    nc.compile()
    _NC_CACHE = nc
    return nc


def _body(nc, tc, mybir, bass, xT, wqk, wv, wo, cosT, sinT, maskm, selm, out):
    dt = mybir.dt
    f32 = dt.float32
    bf = dt.bfloat16
    EXP = mybir.ActivationFunctionType.Exp

    from contextlib import ExitStack
    ctx = ExitStack()
    with ctx:
        consts = ctx.enter_context(tc.tile_pool(name="consts", bufs=1))
        persist = ctx.enter_context(tc.tile_pool(name="persist", bufs=1))
        ropet = ctx.enter_context(tc.tile_pool(name="ropet", bufs=6))
        prpool = ctx.enter_context(tc.tile_pool(name="prpool", bufs=8))
        recpool = ctx.enter_context(tc.tile_pool(name="recpool", bufs=2))
        stpool = ctx.enter_context(tc.tile_pool(name="stpool", bufs=4))
        # PSUM budget: big(2 slots x 2 banks) + acc(2 x 1) + rs(1) + op(1) = 8
        psbig = ctx.enter_context(tc.tile_pool(name="psbig", bufs=2, space="PSUM"))
        psacc = ctx.enter_context(tc.tile_pool(name="psacc", bufs=2, space="PSUM"))
        psrs = ctx.enter_context(tc.tile_pool(name="psrs", bufs=1, space="PSUM"))
        psop = ctx.enter_context(tc.tile_pool(name="psop", bufs=1, space="PSUM"))

        # ---- PE warmup: run throwaway matmuls during the input-DMA wait so
        # the HAM clock gate is at 8/8 (2.4 GHz) by the time real work lands.
        warm = consts.tile([128, 512], bf, tag="warm")
        nc.vector.memset(warm, 0.0)
        wps = psop.tile([128, 512], f32, tag="op", name="warmps")
        for _ in range(14):
            nc.tensor.matmul(wps, lhsT=warm[:, 0:128], rhs=warm,
                             start=True, stop=True)

        # ---- constant/persistent SBUF loads -------------------------------
        # Batched big-tile DMAs (the per-DMA issue on the sync queue costs
        # ~0.6us, so few big transfers beat many small ones), ordered so the
        # first projection chunk can start as early as possible.
        xr = xT.rearrange("(k p) s -> p k s", k=NKT)
        xt_all = consts.tile([128, NKT, S], bf, tag="xt")
        wqk_all = consts.tile([128, NKT, 4 * HPC * HALF], bf, tag="wqk")
        wv_all = consts.tile([128, NKT, HPC * DH], bf, tag="wv")
        wo_all = consts.tile([128, 2, D], bf, tag="wo")
        cos = consts.tile([128, S], bf, tag="cos")
        sin = consts.tile([128, S], bf, tag="sin")
        maskt = consts.tile([JT, 2, JT], bf, tag="maskt")
        selt = consts.tile([128, 256], bf, tag="selt")
        # transfer order follows first use: the k-halves of x chunk 2 and
        # wqk first (the prefix projects q2 first; see below), cos/sin next
        # (rope chain), then the remaining x chunks in projection order
        wqr = wqk.rearrange("(k p) f -> p k f", k=NKT)
        h = NKT // 2
        c2 = slice(2 * IB, 3 * IB)
        nc.sync.dma_start(out=xt_all[:, 0:h, c2], in_=xr[:, 0:h, c2])
        nc.sync.dma_start(out=wqk_all[:, 0:h, :], in_=wqr[:, 0:h, :])
        nc.sync.dma_start(out=xt_all[:, h:NKT, c2], in_=xr[:, h:NKT, c2])
        nc.sync.dma_start(out=wqk_all[:, h:NKT, :], in_=wqr[:, h:NKT, :])
        nc.sync.dma_start(out=cos, in_=cosT)
        nc.sync.dma_start(out=sin, in_=sinT)
        for c in (3, 1, 0):
            csl = slice(c * IB, (c + 1) * IB)
            nc.sync.dma_start(out=xt_all[:, :, csl], in_=xr[:, :, csl])
        nc.sync.dma_start(out=wv_all,
                            in_=wv.rearrange("(k p) f -> p k f", k=NKT))
        nc.sync.dma_start(out=wo_all,
                            in_=wo.rearrange("(k p) f -> p k f", k=2))
        nc.sync.dma_start(out=maskt,
                            in_=maskm.rearrange("p (h f) -> p h f", h=2))
        nc.sync.dma_start(out=selt, in_=selm)
        ones = consts.tile([128, 1], bf, tag="ones")
        nc.vector.memset(ones, 1.0)
        xt = [xt_all[:, k, :] for k in range(NKT)]
        wqkt = [wqk_all[:, k, :] for k in range(NKT)]
        wvt = [wv_all[:, k, :] for k in range(NKT)]
        wot = [wo_all[:, k, :] for k in range(2)]

        # persistent activations
        qR = [persist.tile([128, S], bf, tag=f"qR{i}", name=f"qR{i}") for i in range(2)]
        kR = [persist.tile([128, S], bf, tag=f"kR{i}", name=f"kR{i}") for i in range(2)]
        vbuf = persist.tile([128, NJT, HPC * DH], bf, tag="vbuf")
        oT = [persist.tile([128, S], bf, tag=f"oT{i}", name=f"oT{i}") for i in range(2)]

        # staging buffers for pipelined q/k projection (cast out of PSUM
        # fast, rope later from SBUF so the score ring slot frees quickly)
        kst = persist.tile([128, 4, 256], bf, tag="kst")
        kstf = kst.rearrange("p g f -> p (g f)")
        qst = persist.tile([128, 4, 256], bf, tag="qst")
        qstf = qst.rearrange("p g f -> p (g f)")

        # ---- projection emitters ------------------------------------------
        # wqk cols: [0:128]=q even dims (x1), [128:256]=q odd (x2),
        #           [256:384]=k even, [384:512]=k odd  (4 heads x 32 each)
        def emit_proj_full(ic, r1, r2, fbase):
            """Prefix-time q/k projection + rope straight from PSUM."""
            isl = slice(ic * IB, (ic + 1) * IB)
            pq = psbig.tile([128, 2 * IB], f32, tag="big", name="pq")
            for half in range(2):
                fo = fbase + half * 128
                for k in range(NKT):
                    nc.tensor.matmul(
                        pq[:, half * IB:(half + 1) * IB],
                        lhsT=wqkt[k][:, fo:fo + 128],
                        rhs=xt[k][:, isl],
                        start=(k == 0), stop=(k == NKT - 1))
            # rope: r1 = x1*cos - x2*sin ; r2 = x1*sin + x2*cos
            t1 = ropet.tile([128, IB], bf, tag="rt")
            t2 = ropet.tile([128, IB], bf, tag="rt")
            t3 = ropet.tile([128, IB], bf, tag="rt")
            t4 = ropet.tile([128, IB], bf, tag="rt")
            nc.vector.tensor_mul(t1, pq[:, 0:IB], cos[:, isl])
            nc.vector.tensor_mul(t2, pq[:, IB:2 * IB], sin[:, isl])
            nc.vector.tensor_mul(t3, pq[:, 0:IB], sin[:, isl])
            nc.vector.tensor_mul(t4, pq[:, IB:2 * IB], cos[:, isl])
            nc.vector.tensor_sub(r1[:, isl], t1, t2)
            nc.vector.tensor_add(r2[:, isl], t3, t4)

        def emit_proj_v(jt, pool):
            """One v j-tile ([seq, dim] layout) through the given PSUM pool.
            The PSUM->SBUF copy rides the ScalarE queue (idle slack there;
            the DVE queue would delay the ring-slot release)."""
            vp = pool.tile([128, 2 * IB] if pool is psbig else [128, IB],
                           f32, tag="big" if pool is psbig else "acc",
                           name="vp")
            for k in range(NKT):
                nc.tensor.matmul(
                    vp[:, 0:HPC * DH],
                    lhsT=xt[k][:, jt * 128:(jt + 1) * 128],
                    rhs=wvt[k],
                    start=(k == 0), stop=(k == NKT - 1))
            nc.scalar.copy(out=vbuf[:, jt, :], in_=vp[:, 0:HPC * DH])

        def emit_proj_quarter(c, g, which):
            """In-loop q/k-projection quarter: 8 matmuls into a score-ring
            slot plus a fast ScalarE cast to the SBUF staging tile."""
            stage, fbase = (qst, 0) if which == "q" else (kst, 256)
            half, part = g // 2, g % 2
            fo = fbase + half * 128
            q_lo = c * IB + part * 256
            pq = psbig.tile([128, 2 * IB], f32, tag="big", name="pqk")
            for k in range(NKT):
                nc.tensor.matmul(
                    pq[:, 0:256],
                    lhsT=wqkt[k][:, fo:fo + 128],
                    rhs=xt[k][:, q_lo:q_lo + 256],
                    start=(k == 0), stop=(k == NKT - 1))
            nc.scalar.copy(out=stage[:, g, :], in_=pq[:, 0:256])

        def emit_rope_pipe(c, step, which):
            """One third of the rope for a pipelined q/k chunk (spread
            across iterations to avoid DVE-queue bursts)."""
            stf = qstf if which == "q" else kstf
            r1, r2 = (qR[0], qR[1]) if which == "q" else (kR[0], kR[1])
            hold = rope_hold[which]
            isl = slice(c * IB, (c + 1) * IB)
            x1 = stf[:, 0:IB]
            x2 = stf[:, IB:2 * IB]
            if step == 0:
                t1 = ropet.tile([128, IB], bf, tag="rt")
                t2 = ropet.tile([128, IB], bf, tag="rt")
                nc.vector.tensor_mul(t1, x1, cos[:, isl])
                nc.vector.tensor_mul(t2, x2, sin[:, isl])
                hold[0:2] = [t1, t2]
            elif step == 1:
                t3 = ropet.tile([128, IB], bf, tag="rt")
                t4 = ropet.tile([128, IB], bf, tag="rt")
                nc.vector.tensor_mul(t3, x1, sin[:, isl])
                nc.vector.tensor_mul(t4, x2, cos[:, isl])
                hold[2:4] = [t3, t4]
            else:
                nc.vector.tensor_sub(r1[:, isl], hold[0], hold[1])
                nc.vector.tensor_add(r2[:, isl], hold[2], hold[3])
        rope_hold = {"q": [None] * 4, "k": [None] * 4}

        # ---- prefix: all q chunks + k/v for block 0 -----------------------
        # chunk order q2,q3,q1,q0,k0: the score ring's first two slots then
        # WAR on the ropes of q0/k0 (needed for block 0's data anyway)
        # instead of on unrelated late-chunk ropes
        for ic in (2, 3, 1, 0):
            emit_proj_full(ic, qR[0], qR[1], 0)
        emit_proj_full(0, kR[0], kR[1], 256)
        for jt in range(4):
            emit_proj_v(jt, psacc)

        # ---- phase 2: attention per query block ---------------------------
        # output-projection units are spread one-per-j-iteration into the
        # NEXT block's attention loop so they fill PE gaps instead of
        # forming a serial chain at block boundaries
        pending_op = []

        def emit_op(ic, mc, pool=None, tag="op"):
            icsl = slice(ic * 128, (ic + 1) * 128)
            msl = slice(mc * 512, (mc + 1) * 512)
            op = (pool or psop).tile([128, 512], f32, tag=tag, name="op")
            nc.tensor.matmul(op, lhsT=oT[0][:, icsl], rhs=wot[0][:, msl],
                             start=True, stop=False)
            nc.tensor.matmul(op, lhsT=oT[1][:, icsl], rhs=wot[1][:, msl],
                             start=False, stop=True)
            st = stpool.tile([128, 512], bf, tag="st", name="st")
            nc.vector.tensor_copy(st, op)
            nc.sync.dma_start(out=out[icsl, msl], in_=st)

        def emit_scores(b, jt):
            """Scores matmuls + exp for one (query-block, key-tile) pair."""
            isl = slice(b * IB, (b + 1) * IB)
            jsl = slice(jt * JT, (jt + 1) * JT)
            sc1 = psbig.tile([128, 2 * IB], f32, tag="big", name="sc1")
            sc2 = psbig.tile([128, 2 * IB], f32, tag="big", name="sc2")
            for ph in range(2):
                for h in range(4):
                    dst = sc1 if h < 2 else sc2
                    col = (h % 2) * IB
                    nc.tensor.matmul(
                        dst[:, col:col + IB],
                        lhsT=kR[ph][32 * h:32 * h + 32, jsl],
                        rhs=qR[ph][32 * h:32 * h + 32, isl],
                        start=(ph == 0), stop=(ph == 1),
                        tile_position=(32 * h, 0))
            pr1 = prpool.tile([128, 2, IB], bf, tag="pr", name="pr1")
            pr2 = prpool.tile([128, 2, IB], bf, tag="pr", name="pr2")
            delta = jt * JT - b * IB
            for (sc, pr) in ((sc1, pr1), (sc2, pr2)):
                scv = sc.rearrange("p (h f) -> p h f", h=2)
                if delta < 0:
                    nc.scalar.activation(pr, sc, EXP)
                else:
                    # exp only the possibly-valid region (pv/rs matmuls
                    # read the same subrange, so the rest of pr can stay
                    # garbage), mask the 128-wide diagonal block
                    nc.scalar.activation(pr[:, :, delta:IB],
                                         scv[:, :, delta:IB], EXP)
                    nc.vector.tensor_mul(pr[:, :, delta:delta + JT],
                                         pr[:, :, delta:delta + JT],
                                         maskt)
            return pr1, pr2

        def emit_pv(b, jt, pr1, pr2, pv1, pv2, rs, last):
            """PV + row-sum matmuls for one (b, jt); lags scores by 1 iter."""
            vj = vbuf[:, jt, :]
            st0 = (jt == 0)
            d0 = max(jt * JT - b * IB, 0)
            nc.tensor.matmul(pv1[0:64, d0:IB], lhsT=vj[:, 0:64],
                             rhs=pr1[:, 0, d0:IB], start=st0, stop=last,
                             skip_group_check=True, tile_position=(0, 0))
            nc.tensor.matmul(pv1[64:128, d0:IB], lhsT=vj[:, 64:128],
                             rhs=pr1[:, 1, d0:IB], start=st0, stop=last,
                             skip_group_check=True, tile_position=(0, 64))
            nc.tensor.matmul(pv2[0:64, d0:IB], lhsT=vj[:, 128:192],
                             rhs=pr2[:, 0, d0:IB], start=st0, stop=last,
                             skip_group_check=True, tile_position=(0, 0))
            nc.tensor.matmul(pv2[64:128, d0:IB], lhsT=vj[:, 192:256],
                             rhs=pr2[:, 1, d0:IB], start=st0, stop=last,
                             skip_group_check=True, tile_position=(0, 64))
            nc.tensor.matmul(rs[0:1, d0:IB], lhsT=ones,
                             rhs=pr1[:, 0, d0:IB], start=st0, stop=last,
                             skip_group_check=True, tile_position=(0, 0))
            nc.tensor.matmul(rs[32:33, d0:IB], lhsT=ones,
                             rhs=pr1[:, 1, d0:IB], start=st0, stop=last,
                             skip_group_check=True, tile_position=(0, 32))
            nc.tensor.matmul(rs[64:65, d0:IB], lhsT=ones,
                             rhs=pr2[:, 0, d0:IB], start=st0, stop=last,
                             skip_group_check=True, tile_position=(0, 64))
            nc.tensor.matmul(rs[96:97, d0:IB], lhsT=ones,
                             rhs=pr2[:, 1, d0:IB], start=st0, stop=last,
                             skip_group_check=True, tile_position=(0, 96))

        def emit_norm(b, pv1, pv2, rs):
            """Normalization chain: oT = pv * broadcast(1/rs).

            Cast the sum rows to bf16 SBUF (rows 0/32/64/96 hold sums, the
            rest the memset 1.0s), selector-matmul broadcasts r across
            partitions into PSUM (bc1 rows 0-63 <- rs row 0, rows 64-127 <-
            rs row 32, etc.), then reciprocal PSUM->SBUF so the final DVE
            multiply reads only one PSUM operand (pv)."""
            isl = slice(b * IB, (b + 1) * IB)
            rsbf = recpool.tile([128, IB], bf, tag="rsbf")
            nc.vector.tensor_copy(rsbf, rs)
            bc1 = psop.tile([128, IB], f32, tag="op", name="bc1")
            nc.tensor.matmul(bc1, lhsT=selt[:, 0:128], rhs=rsbf,
                             start=True, stop=True)
            bc2 = psrs.tile([128, IB], f32, tag="rs", name="bc2")
            nc.tensor.matmul(bc2, lhsT=selt[:, 128:256], rhs=rsbf,
                             start=True, stop=True)
            rec1 = recpool.tile([128, IB], f32, tag="rec")
            rec2 = recpool.tile([128, IB], f32, tag="rec")
            nc.vector.reciprocal_approx_fast(out=rec1, in_=bc1)
            nc.vector.reciprocal_approx_fast(out=rec2, in_=bc2)
            nc.vector.tensor_mul(oT[0][:, isl], pv1, rec1)
            nc.vector.tensor_mul(oT[1][:, isl], pv2, rec2)
            pending_op.extend((ic, mc) for ic in range(4 * b, 4 * b + 4)
                              for mc in range(2))

        # The jt loop lags pv/rs one iteration behind scores/exp so the PE
        # FIFO always holds runnable score matmuls while the ScalarE works:
        # PE order per iter is [scores(jt), pv(jt-1), op-unit, proj-inserts],
        # and each block's norm chain + accumulator allocation happen after
        # the next block's first scores are already in flight.  k/v
        # projection for block b+1 is pipelined through the score ring as
        # small quarter-units during block b (their column ranges of kR/vbuf
        # are disjoint from what block b reads).
        pend_pvs = []
        norm_args = None
        pv1 = pv2 = rs = None
        ins_backlog = []
        for b in range(NIB):
            njt = (b + 1) * (IB // JT)
            if b + 1 < NIB:
                ins_backlog += ([("pq", b + 1, g, "k") for g in range(4)]
                                + [("pr", b + 1, s, "k") for s in range(3)]
                                + [("v", 4 * (b + 1) + g) for g in range(4)])
            for jt in range(njt):
                pr1, pr2 = emit_scores(b, jt)
                if jt == 0:
                    # block boundary: flush all lagged pv groups of the
                    # previous block, then its norm chain
                    for p in pend_pvs:
                        emit_pv(**p)
                    pend_pvs.clear()
                    if norm_args is not None:
                        emit_norm(*norm_args)
                        norm_args = None
                elif len(pend_pvs) >= 2:
                    emit_pv(**pend_pvs.pop(0))
                if jt == 0:
                    # allocate this block's accumulators only after the
                    # previous block's norm (ring-slot order: rs -> bc2 ->
                    # next rs)
                    pv1 = psacc.tile([128, IB], f32, tag="acc", name="pv1")
                    pv2 = psacc.tile([128, IB], f32, tag="acc", name="pv2")
                    rs = psrs.tile([128, IB], f32, tag="rs", name="rs")
                    nc.vector.memset(rs, 1.0)
                # output-projection pops start at jt>=3 so they never
                # FIFO-block on the previous block's norm chain
                if pending_op and jt >= 3:
                    emit_op(*pending_op.pop(0))
                    if len(pending_op) > (njt - 1 - jt) and pending_op:
                        emit_op(*pending_op.pop(0))
                # projection inserts, spread over the block's iterations
                if jt > 0 and ins_backlog:
                    iters_left = njt - jt
                    n_emit = -(-len(ins_backlog) // iters_left)  # ceil
                    for _ in range(n_emit):
                        ins = ins_backlog.pop(0)
                        if ins[0] == "pq":
                            emit_proj_quarter(ins[1], ins[2], ins[3])
                        elif ins[0] == "pr":
                            emit_rope_pipe(ins[1], ins[2], ins[3])
                        else:
                            emit_proj_v(ins[1], psbig)
                pend_pvs.append(dict(b=b, jt=jt, pr1=pr1, pr2=pr2,
                                     pv1=pv1, pv2=pv2, rs=rs,
                                     last=(jt == njt - 1)))
            norm_args = (b, pv1, pv2, rs)
        for p in pend_pvs:
            emit_pv(**p)
        pend_pvs.clear()
        emit_norm(*norm_args)

        # drain the last block's output projection, alternating PSUM slots
        # (the rs bank is free by now) so the tail pipelines
        di = 0
        while pending_op:
            ic, mc = pending_op.pop(0)
            if di % 2 == 0:
                emit_op(ic, mc)
            else:
                emit_op(ic, mc, pool=psrs, tag="rs")
            di += 1


# ---------------------------------------------------------------------------
# Host-side sharding / unsharding
# ---------------------------------------------------------------------------

def _core_inputs(x, Wqkv, Wo, core):
    """Build the bf16 input map for one core (numpy, cheap)."""
    b = core // GPB
    heads = [HPC * (core % GPB) + j for j in range(HPC)]

    Wq = Wqkv[0 * D:1 * D]
    Wk = Wqkv[1 * D:2 * D]
    Wv = Wqkv[2 * D:3 * D]

    rows_x1 = [h * DH + 2 * t for h in heads for t in range(HALF)]
    rows_x2 = [h * DH + 2 * t + 1 for h in heads for t in range(HALF)]
    rows_v = [h * DH + d for h in heads for d in range(DH)]

    scale = 1.0 / np.sqrt(DH)
    wqk_host = np.concatenate([
        Wq[rows_x1] * scale, Wq[rows_x2] * scale,
        Wk[rows_x1], Wk[rows_x2],
    ], axis=0)                                   # [512, 1024]

    inv = THETA ** (-np.arange(HALF, dtype=np.float64) / HALF)
    ang = np.arange(S, dtype=np.float64)[None, :] * inv[:, None]   # [32, S]
    cos = np.tile(np.cos(ang), (HPC, 1))
    sin = np.tile(np.sin(ang), (HPC, 1))

    tri = (np.arange(JT)[None, :] >= np.arange(JT)[:, None]).astype(np.float32)
    maskm = np.tile(tri, (1, 2))                                   # [128, 256]

    # selector matrices for the partition-broadcast of the softmax
    # denominators: bc[p] = rec[sel_row(p)]
    selm = np.zeros((128, 256), dtype=np.float32)
    selm[0, 0:64] = 1.0       # bc1 rows 0-63   <- rec row 0  (head 0)
    selm[32, 64:128] = 1.0    # bc1 rows 64-127 <- rec row 32 (head 1)
    selm[64, 128:192] = 1.0   # bc2 rows 0-63   <- rec row 64 (head 2)
    selm[96, 192:256] = 1.0   # bc2 rows 64-127 <- rec row 96 (head 3)

    c = lambda a: np.ascontiguousarray(a).astype(BF16)
    return {
        "xT": c(x[b].T),
        "wqk": c(wqk_host.T),
        "wv": c(Wv[rows_v].T),
        "wo": c(Wo[:, rows_v].T),
        "cosT": c(cos),
        "sinT": c(sin),
        "maskm": c(maskm),
        "selm": c(selm),
    }


def _run(x, Wqkv, Wo, trace=False):
    nc = _build_nc()
    from concourse.bass_utils import run_bass_kernel_spmd
    in_maps = [_core_inputs(x, Wqkv, Wo, c) for c in range(NCORES)]
    res = run_bass_kernel_spmd(nc, in_maps, core_ids=list(range(NCORES)),
                               trace=trace)
    parts = [res.results[i]["out"].astype(np.float32) for i in range(NCORES)]
    full = np.stack([sum(parts[0:GPB]), sum(parts[GPB:2 * GPB])], axis=0)
    return full, res


def kernel(x, Wqkv, Wo):
    x = np.asarray(x, dtype=np.float32)
    Wqkv = np.asarray(Wqkv, dtype=np.float32)
    Wo = np.asarray(Wo, dtype=np.float32)
    full, _ = _run(x, Wqkv, Wo, trace=False)
    return full

